# revision 17
# baseline (speedup 1.0000x reference)
"""Trainium2 Bass kernel for nn_Node3DEmbeddingv2 (gnn_message_passing).

Strategy (8 NeuronCores, SPMD, data-parallel over flattened (batch, query-row)):
  - 1536 query rows split into 8 x 192 (batch-aligned: 4 cores per batch).
  - Per core, per 32-row group: pairwise deltas vs all 768 keys on DVE,
    squared, reduced over xyz via a block-replicating matmul -> d^2 (PSUM),
    ACT sqrt -> d (replicated 3x across partitions).
  - d is split into 3 bf16 components (hi/mid/lo, exact to f32 precision);
    a [3,128]-ones bf16 matmul broadcasts each row's 768 distances across
    all 128 gaussian-channel partitions at full PE rate (f32 accumulate).
  - One ScalarE op per row computes the whole Gaussian:
      Derivative_Erf(scale_k * d + bias_k) = 2/sqrt(pi) * exp(-((d-m_k)/s_k)^2/2)
    with accum_out producing the sum over the 768 keys for free.
    (Fallback path: Square + Exp, two ACT passes, if the table is inaccurate.)
  - Channel constants 1/(sqrt(2 pi) s_k) (and the table constant) are applied
    post-reduction on the tiny [128, 192] summed tensor.
  - feature_proj MLP (gelu between two matmuls) on device; PE transposes the
    [E, rows] result back to row-major, adds the host-computed angle/time
    embedding tail, DMAs out [192, 512] per core.
  - Host (numpy, negligible): angle MLP, sinusoidal time embedding MLP,
    masking, per-core input prep; all heavy compute is on-device.
"""

import math

import numpy as np

# Problem constants (hardcoded per the task contract).
B, N, K, E = 2, 768, 128, 512
INTER = E // 2
NCORES = 8
RPC = (B * N) // NCORES  # 192 rows per core
GR = 32                  # rows per group
NGROUPS = RPC // GR      # 6 groups per core
PI_REF = 3.14159         # matches reference's gaussian constant

# Derivative_Erf table semantics: d/dx erf(x) = 2/sqrt(pi) * exp(-x^2).
# DERF_INV is the factor that converts the table output back to exp(-x^2).
DERF_INV = math.sqrt(math.pi) / 2.0

# Set to True to use the Square+Exp fallback instead of Derivative_Erf.
USE_FALLBACK_EXP = False
# Simulator/testing hook: replace Derivative_Erf by another func (e.g. Square).
_FUNC_OVERRIDE = None

_COMPILED = {}


def _build_nc(use_fallback=None, func_override=None, gelu_override=None):
    import concourse.bass as bass
    import concourse.bacc as bacc
    from concourse import mybir
    from concourse.tile import TileContext

    if use_fallback is None:
        use_fallback = USE_FALLBACK_EXP
    f32 = mybir.dt.float32
    bf16 = mybir.dt.bfloat16
    AF = mybir.ActivationFunctionType

    nc = bacc.Bacc("TRN2", target_bir_lowering=False)

    # DRAM I/O (per-core values supplied via in_maps).
    posT = nc.dram_tensor("posT", [3, N], f32, kind="ExternalInput")
    qscal = nc.dram_tensor("qscal", [96, NGROUPS], f32, kind="ExternalInput")
    blk3 = nc.dram_tensor("blk3", [96, GR], f32, kind="ExternalInput")
    esc = nc.dram_tensor("esc", [K, 1], f32, kind="ExternalInput")
    ebi = nc.dram_tensor("ebi", [K, 1], f32, kind="ExternalInput")
    postc = nc.dram_tensor("postc", [K, 1], f32, kind="ExternalInput")
    w1 = nc.dram_tensor("w1", [K, K], f32, kind="ExternalInput")
    w2 = nc.dram_tensor("w2", [K, INTER], f32, kind="ExternalInput")
    ident = nc.dram_tensor("ident", [128, 128], f32, kind="ExternalInput")
    rest = nc.dram_tensor("rest", [RPC, E], f32, kind="ExternalInput")
    out = nc.dram_tensor("out", [RPC, E], f32, kind="ExternalOutput")

    with TileContext(nc) as tc:
        with tc.tile_pool(name="sb", bufs=1) as sb:
            # ---- constant loads ----
            pos_rep = sb.tile([96, N], f32, tag="pos_rep")
            nc.sync.dma_start(
                out=pos_rep,
                in_=bass.AP(tensor=posT, offset=0, ap=[[0, 32], [N, 3], [1, N]]),
            )
            q_sb = sb.tile([96, NGROUPS], f32, tag="q_sb")
            nc.sync.dma_start(out=q_sb, in_=qscal[:, :])
            blk_sb = sb.tile([96, GR], f32, tag="blk_sb")
            nc.sync.dma_start(out=blk_sb, in_=blk3[:, :])
            esc_sb = sb.tile([K, 1], f32, tag="esc_sb")
            nc.sync.dma_start(out=esc_sb, in_=esc[:, :])
            ebi_sb = sb.tile([K, 1], f32, tag="ebi_sb")
            nc.sync.dma_start(out=ebi_sb, in_=ebi[:, :])
            postc_sb = sb.tile([K, 1], f32, tag="postc_sb")
            nc.sync.dma_start(out=postc_sb, in_=postc[:, :])
            w1_sb = sb.tile([K, K], f32, tag="w1_sb")
            nc.sync.dma_start(out=w1_sb, in_=w1[:, :])
            w2_sb = sb.tile([K, INTER], f32, tag="w2_sb")
            nc.sync.dma_start(out=w2_sb, in_=w2[:, :])
            id_sb = sb.tile([128, 128], f32, tag="id_sb")
            nc.sync.dma_start(out=id_sb, in_=ident[:, :])
            ones3 = sb.tile([3, 128], bf16, tag="ones3")
            nc.vector.memset(ones3, 1.0)

            S = sb.tile([K, RPC], f32, tag="S")

            # Collapse the many input-DMA queue semaphores into one point so
            # downstream consumers never need more waits than the instruction
            # encoding allows.
            tc.strict_bb_all_engine_barrier()

            # ---- phase A: distances + bf16 splits for all 6 groups ----
            split_tiles = []
            with tc.tile_pool(name="psA", bufs=1, space="PSUM") as psA:
                for g in range(NGROUPS):
                    delta = sb.tile([96, N], f32, tag="delta", bufs=2)
                    nc.vector.tensor_scalar(
                        out=delta,
                        in0=pos_rep,
                        scalar1=q_sb[:, g : g + 1],
                        scalar2=None,
                        op0=mybir.AluOpType.subtract,
                    )
                    nc.vector.tensor_mul(delta, delta, delta)
                    psum_d2 = psA.tile([GR, N], f32, tag="d2", bufs=2)
                    nc.tensor.matmul(
                        psum_d2[:, 0:512], blk_sb, delta[:, 0:512],
                        start=True, stop=True,
                    )
                    nc.tensor.matmul(
                        psum_d2[:, 512:N], blk_sb, delta[:, 512:N],
                        start=True, stop=True,
                    )
                    d_sb = sb.tile([GR, N], f32, tag=f"d{g}")
                    nc.scalar.sqrt(d_sb, psum_d2)
                    # exact 3-way bf16 split: hi + mid + lo == d (f32 precision)
                    dh = sb.tile([GR, N], bf16, tag=f"dh{g}")
                    nc.gpsimd.tensor_copy(dh, d_sb)
                    r1 = sb.tile([GR, N], f32, tag="r1", bufs=2)
                    nc.vector.tensor_sub(r1, d_sb, dh)
                    dm = sb.tile([GR, N], bf16, tag=f"dm{g}")
                    nc.gpsimd.tensor_copy(dm, r1)
                    r2 = sb.tile([GR, N], f32, tag="r2", bufs=2)
                    nc.vector.tensor_sub(r2, r1, dm)
                    dl = sb.tile([GR, N], bf16, tag=f"dl{g}")
                    nc.gpsimd.tensor_copy(dl, r2)
                    split_tiles.append((dh, dm, dl))

            # ---- phase B: broadcast + gaussian + key-sum per row ----
            derf_func = AF.Derivative_Erf
            if func_override is not None:
                derf_func = func_override
            with tc.tile_pool(name="psB", bufs=1, space="PSUM") as psB:
                for g in range(NGROUPS):
                    # flatten the group's rows onto partitions 0..2:
                    # M_flat[c, a*N + j] = component c of row a at key j
                    M_flat = sb.tile([3, GR * N], bf16, tag="mflat", bufs=2)
                    for c, comp in enumerate(split_tiles[g]):
                        nc.gpsimd.dma_start(
                            out=M_flat[c : c + 1, :].rearrange(
                                "p (a j) -> p a j", a=GR
                            ),
                            in_=comp[:, :],
                        )
                    for a in range(0, GR, 2):
                        r = g * GR + a
                        # two rows per PSUM tile; matmuls split on bank
                        # boundaries (512 f32 per 2KB bank)
                        psum_db = psB.tile([K, 2, N], f32, tag="db", bufs=2)
                        flat = psum_db.rearrange("k a j -> k (a j)")
                        for lo, hi in ((0, 512), (512, 768), (768, 1024), (1024, 1536)):
                            src = M_flat[0:3, a * N + lo : a * N + hi]
                            nc.tensor.matmul(
                                flat[:, lo:hi], ones3, src,
                                start=True, stop=True,
                            )
                        if not use_fallback:
                            gsc = sb.tile([K, 2, N], f32, tag="gsc", bufs=3)
                            nc.scalar.activation(
                                out=gsc,
                                in_=psum_db,
                                func=derf_func,
                                bias=ebi_sb,
                                scale=esc_sb,
                            )
                        else:
                            zsq = sb.tile([K, 2, N], f32, tag="zsq", bufs=3)
                            nc.scalar.activation(
                                out=zsq, in_=psum_db,
                                func=AF.Square, bias=ebi_sb, scale=esc_sb,
                            )
                            gsc = sb.tile([K, 2, N], f32, tag="gsc", bufs=3)
                            nc.scalar.activation(
                                out=gsc, in_=zsq,
                                func=AF.Exp, bias=postc_sb, scale=-0.5,
                            )
                        # key-axis sum: per-row tensor_scalar with accum_out
                        # (2x_1P-eligible, unlike 1x tensor_reduce)
                        for q in range(2):
                            nc.vector.tensor_scalar(
                                out=gsc[:, q, :],
                                in0=gsc[:, q, :],
                                scalar1=0.0,
                                scalar2=None,
                                op0=mybir.AluOpType.add,
                                op1=mybir.AluOpType.add,
                                accum_out=S[:, r + q : r + q + 1],
                            )

            # ---- phase C: channel constants + feature_proj MLP + output ----
            with tc.tile_pool(name="psC", bufs=1, space="PSUM") as psC:
                if not use_fallback:
                    nc.vector.tensor_scalar_mul(S, S, postc_sb)
                psum_h = psC.tile([K, RPC], f32, tag="mlp", bufs=2)
                nc.tensor.matmul(psum_h, w1_sb, S, start=True, stop=True)
                h_sb = sb.tile([K, RPC], f32, tag="h_sb")
                gelu_func = AF.Gelu if gelu_override is None else gelu_override
                nc.scalar.activation(h_sb, psum_h, gelu_func)
                o_sb = sb.tile([128, 2, RPC], f32, tag="o_sb")
                for e in range(2):
                    psum_o = psC.tile([128, RPC], f32, tag="mlp", bufs=2)
                    nc.tensor.matmul(
                        psum_o, w2_sb[:, 128 * e : 128 * (e + 1)], h_sb,
                        start=True, stop=True,
                    )
                    nc.vector.tensor_copy(o_sb[:, e, :], psum_o)
                for t in range(2):
                    out_sb = sb.tile([96, E], f32, tag=f"out{t}")
                    nc.gpsimd.dma_start(
                        out=out_sb, in_=rest[96 * t : 96 * (t + 1), :]
                    )
                    for e in range(2):
                        psum_t = psC.tile([96, 128], f32, tag="tr", bufs=2)
                        nc.tensor.transpose(
                            psum_t, o_sb[:, e, 96 * t : 96 * (t + 1)], id_sb
                        )
                        nc.vector.tensor_add(
                            out_sb[:, 128 * e : 128 * (e + 1)],
                            out_sb[:, 128 * e : 128 * (e + 1)],
                            psum_t,
                        )
                    nc.sync.dma_start(
                        out=out[96 * t : 96 * (t + 1), :], in_=out_sb
                    )

    nc.compile()
    return nc


# ---------------- host-side reference tails (numpy, f32) ----------------

def _erf_np(x):
    try:
        from scipy.special import erf
        return erf(x).astype(np.float32)
    except ImportError:
        f = np.frompyfunc(math.erf, 1, 1)
        return f(x.astype(np.float64)).astype(np.float32)


def _gelu_np(x):
    x = x.astype(np.float32)
    return (x * 0.5 * (1.0 + _erf_np(x / np.float32(math.sqrt(2.0))))).astype(
        np.float32
    )


def _silu_np(x):
    x = x.astype(np.float32)
    return (x / (1.0 + np.exp(-x))).astype(np.float32)


def _timestep_emb_np(t, dim):
    half = dim // 2
    freqs = np.exp(
        -np.log(10000.0) * np.arange(half, dtype=np.float32) / np.float32(half)
    ).astype(np.float32)
    a = t.astype(np.float32)[:, None] * freqs[None, :]
    return np.concatenate([np.sin(a), np.cos(a)], axis=-1).astype(np.float32)


def _host_tails(angle, mask_pos, time_pos, ang_w1, ang_w2, t_w1, t_b1, t_w2, t_b2):
    """rest[b, n, :] with rest[..., :INTER] = time_emb[..., :INTER] and
    rest[..., INTER:] = ang_f + time_emb[..., INTER:]."""
    angle = np.asarray(angle, np.float32)
    ang = np.where(np.isposinf(angle), np.float32(0.0), angle).astype(np.float32)
    ang_f = _gelu_np(ang @ np.asarray(ang_w1, np.float32)) @ np.asarray(
        ang_w2, np.float32
    )  # [B, N, INTER]

    def time_mlp(t):
        e = _timestep_emb_np(t, E)
        h = _silu_np(e @ np.asarray(t_w1, np.float32) + np.asarray(t_b1, np.float32))
        return (h @ np.asarray(t_w2, np.float32) + np.asarray(t_b2, np.float32)).astype(
            np.float32
        )

    tp = np.asarray(time_pos)
    te = time_mlp(tp)[:, None, :]                 # [B, 1, E]
    t0e = time_mlp(np.zeros_like(tp))[:, None, :]
    mask = np.asarray(mask_pos, bool)             # [B, N, 1]
    time_emb = np.where(mask, te, t0e).astype(np.float32)  # [B, N, E]

    rest = time_emb.copy()
    rest[..., INTER:] += ang_f.astype(np.float32)
    return rest.astype(np.float32)


def _prep_in_maps(pos, angle, padding_mask, mask_pos, time_pos,
                  means, stds, fp_w1, fp_w2, ang_w1, ang_w2,
                  t_w1, t_b1, t_w2, t_b2, use_fallback=None):
    if use_fallback is None:
        use_fallback = USE_FALLBACK_EXP
    pos = np.asarray(pos, np.float32)
    pad = np.asarray(padding_mask, bool)

    s = (np.abs(np.asarray(stds, np.float32)) + np.float32(0.01)).astype(np.float32)
    m = np.asarray(means, np.float32)
    inv_s = (np.float32(1.0) / s).astype(np.float32)
    if not use_fallback:
        # Derivative_Erf(x) with x = (d - m)/(s*sqrt(2))
        esc_v = (inv_s / np.float32(math.sqrt(2.0))).astype(np.float32)
        ebi_v = (-m * esc_v).astype(np.float32)
        postc_v = (
            np.float32(DERF_INV) / (np.float32(math.sqrt(2.0 * PI_REF)) * s)
        ).astype(np.float32)
    else:
        # Square then Exp(-0.5 z^2 + log c)
        esc_v = inv_s.astype(np.float32)
        ebi_v = (-m * inv_s).astype(np.float32)
        postc_v = np.log(
            np.float32(1.0) / (np.float32(math.sqrt(2.0 * PI_REF)) * s)
        ).astype(np.float32)

    blk3 = np.zeros((96, GR), np.float32)
    for p in range(96):
        blk3[p, p // 3] = 1.0

    rest = _host_tails(
        angle, mask_pos, time_pos, ang_w1, ang_w2, t_w1, t_b1, t_w2, t_b2
    )

    ident = np.eye(128, dtype=np.float32)
    w1_v = np.asarray(fp_w1, np.float32)
    w2_v = np.asarray(fp_w2, np.float32)

    in_maps = []
    for c in range(NCORES):
        b = c // (NCORES // B)
        r0 = (c % (NCORES // B)) * RPC
        posT = pos[b].T.copy()  # [3, N]
        if pad[b].any():
            posT[:, pad[b]] = np.float32(1.0e6)
        qscal = np.empty((96, NGROUPS), np.float32)
        for g in range(NGROUPS):
            rows = pos[b, r0 + g * GR : r0 + (g + 1) * GR, :]  # [32, 3]
            qscal[:, g] = rows.reshape(-1)
        in_maps.append(
            {
                "posT": np.ascontiguousarray(posT, np.float32),
                "qscal": qscal,
                "blk3": blk3,
                "esc": esc_v.reshape(K, 1),
                "ebi": ebi_v.reshape(K, 1),
                "postc": postc_v.reshape(K, 1),
                "w1": w1_v,
                "w2": w2_v,
                "ident": ident,
                "rest": np.ascontiguousarray(rest[b, r0 : r0 + RPC, :], np.float32),
            }
        )
    return in_maps


def kernel(pos, angle, node_type_edge, padding_mask, mask_aa, mask_pos, time_pos,
           means, stds, fp_w1, fp_w2, ang_w1, ang_w2, t_w1, t_b1, t_w2, t_b2):
    from concourse.bass_utils import run_bass_kernel_spmd

    key = ("nc", USE_FALLBACK_EXP, _FUNC_OVERRIDE)
    if key not in _COMPILED:
        _COMPILED[key] = _build_nc(func_override=_FUNC_OVERRIDE)
    nc = _COMPILED[key]

    in_maps = _prep_in_maps(
        pos, angle, padding_mask, mask_pos, time_pos, means, stds,
        fp_w1, fp_w2, ang_w1, ang_w2, t_w1, t_b1, t_w2, t_b2,
    )
    res = run_bass_kernel_spmd(nc, in_maps, core_ids=list(range(NCORES)))
    outs = [np.asarray(res.results[c]["out"], np.float32) for c in range(NCORES)]
    full = np.concatenate(outs, axis=0).reshape(B, N, E)
    return full


# revision 20
# speedup vs baseline: 1.2056x; 1.2056x over previous
"""Trainium2 Bass kernel for nn_Node3DEmbeddingv2 (gnn_message_passing).

Strategy (8 NeuronCores, SPMD, data-parallel over flattened (batch, query-row)):
  - 1536 query rows split into 8 x 192 (batch-aligned: 4 cores per batch).
  - Per core, per 32-row group: pairwise deltas vs all 768 keys on DVE,
    squared, reduced over xyz via a block-replicating matmul -> d^2 (PSUM),
    ACT sqrt -> d (replicated 3x across partitions).
  - d is split into 3 bf16 components (hi/mid/lo, exact to f32 precision);
    a [3,128]-ones bf16 matmul broadcasts each row's 768 distances across
    all 128 gaussian-channel partitions at full PE rate (f32 accumulate).
  - One ScalarE op per row computes the whole Gaussian:
      Derivative_Erf(scale_k * d + bias_k) = 2/sqrt(pi) * exp(-((d-m_k)/s_k)^2/2)
    with accum_out producing the sum over the 768 keys for free.
    (Fallback path: Square + Exp, two ACT passes, if the table is inaccurate.)
  - Channel constants 1/(sqrt(2 pi) s_k) (and the table constant) are applied
    post-reduction on the tiny [128, 192] summed tensor.
  - feature_proj MLP (gelu between two matmuls) on device; PE transposes the
    [E, rows] result back to row-major, adds the host-computed angle/time
    embedding tail, DMAs out [192, 512] per core.
  - Host (numpy, negligible): angle MLP, sinusoidal time embedding MLP,
    masking, per-core input prep; all heavy compute is on-device.
"""

import math

import numpy as np

# Problem constants (hardcoded per the task contract).
B, N, K, E = 2, 768, 128, 512
INTER = E // 2
NCORES = 8
RPC = (B * N) // NCORES  # 192 rows per core
GR = 32                  # rows per group
NGROUPS = RPC // GR      # 6 groups per core
PI_REF = 3.14159         # matches reference's gaussian constant

# Derivative_Erf table semantics: d/dx erf(x) = 2/sqrt(pi) * exp(-x^2).
# DERF_INV is the factor that converts the table output back to exp(-x^2).
DERF_INV = math.sqrt(math.pi) / 2.0

# Set to True to use the Square+Exp fallback instead of Derivative_Erf.
USE_FALLBACK_EXP = False
# Simulator/testing hook: replace Derivative_Erf by another func (e.g. Square).
_FUNC_OVERRIDE = None

_COMPILED = {}


def _build_nc(use_fallback=None, func_override=None, gelu_override=None):
    import concourse.bass as bass
    import concourse.bacc as bacc
    from concourse import mybir
    from concourse.tile import TileContext

    if use_fallback is None:
        use_fallback = USE_FALLBACK_EXP
    f32 = mybir.dt.float32
    bf16 = mybir.dt.bfloat16
    AF = mybir.ActivationFunctionType

    nc = bacc.Bacc("TRN2", target_bir_lowering=False)

    # DRAM I/O (per-core values supplied via in_maps).
    posT = nc.dram_tensor("posT", [3, N], f32, kind="ExternalInput")
    qscal = nc.dram_tensor("qscal", [96, NGROUPS], f32, kind="ExternalInput")
    blk3 = nc.dram_tensor("blk3", [96, GR], f32, kind="ExternalInput")
    esc = nc.dram_tensor("esc", [K, 1], f32, kind="ExternalInput")
    ebi = nc.dram_tensor("ebi", [K, 1], f32, kind="ExternalInput")
    postc = nc.dram_tensor("postc", [K, 1], f32, kind="ExternalInput")
    w1 = nc.dram_tensor("w1", [K, K], f32, kind="ExternalInput")
    w2 = nc.dram_tensor("w2", [K, INTER], f32, kind="ExternalInput")
    ident = nc.dram_tensor("ident", [128, 128], f32, kind="ExternalInput")
    rest = nc.dram_tensor("rest", [RPC, E], f32, kind="ExternalInput")
    out = nc.dram_tensor("out", [RPC, E], f32, kind="ExternalOutput")

    with TileContext(nc) as tc:
        with tc.tile_pool(name="sb", bufs=1) as sb:
            # ---- constant loads ----
            pos_rep = sb.tile([96, N], f32, tag="pos_rep")
            nc.sync.dma_start(
                out=pos_rep,
                in_=bass.AP(tensor=posT, offset=0, ap=[[0, 32], [N, 3], [1, N]]),
            )
            q_sb = sb.tile([96, NGROUPS], f32, tag="q_sb")
            nc.sync.dma_start(out=q_sb, in_=qscal[:, :])
            blk_sb = sb.tile([96, GR], f32, tag="blk_sb")
            nc.sync.dma_start(out=blk_sb, in_=blk3[:, :])
            esc_sb = sb.tile([K, 1], f32, tag="esc_sb")
            nc.sync.dma_start(out=esc_sb, in_=esc[:, :])
            ebi_sb = sb.tile([K, 1], f32, tag="ebi_sb")
            nc.sync.dma_start(out=ebi_sb, in_=ebi[:, :])
            postc_sb = sb.tile([K, 1], f32, tag="postc_sb")
            nc.sync.dma_start(out=postc_sb, in_=postc[:, :])
            w1_sb = sb.tile([K, K], f32, tag="w1_sb")
            nc.sync.dma_start(out=w1_sb, in_=w1[:, :])
            w2_sb = sb.tile([K, INTER], f32, tag="w2_sb")
            nc.sync.dma_start(out=w2_sb, in_=w2[:, :])
            id_sb = sb.tile([128, 128], f32, tag="id_sb")
            nc.sync.dma_start(out=id_sb, in_=ident[:, :])
            ones3 = sb.tile([3, 128], bf16, tag="ones3")
            nc.vector.memset(ones3, 1.0)

            S = sb.tile([K, RPC], f32, tag="S")

            # Collapse the many input-DMA queue semaphores into one point so
            # downstream consumers never need more waits than the instruction
            # encoding allows.
            tc.strict_bb_all_engine_barrier()

            # ---- phase A: distances + bf16 splits for all 6 groups ----
            split_tiles = []
            with tc.tile_pool(name="psA", bufs=1, space="PSUM") as psA:
                for g in range(NGROUPS):
                    delta = sb.tile([96, N], f32, tag="delta", bufs=2)
                    nc.vector.tensor_scalar(
                        out=delta,
                        in0=pos_rep,
                        scalar1=q_sb[:, g : g + 1],
                        scalar2=None,
                        op0=mybir.AluOpType.subtract,
                    )
                    nc.vector.tensor_mul(delta, delta, delta)
                    psum_d2 = psA.tile([GR, N], f32, tag="d2", bufs=2)
                    nc.tensor.matmul(
                        psum_d2[:, 0:512], blk_sb, delta[:, 0:512],
                        start=True, stop=True,
                    )
                    nc.tensor.matmul(
                        psum_d2[:, 512:N], blk_sb, delta[:, 512:N],
                        start=True, stop=True,
                    )
                    d_sb = sb.tile([GR, N], f32, tag=f"d{g}")
                    nc.scalar.sqrt(d_sb, psum_d2)
                    # exact 3-way bf16 split: hi + mid + lo == d (f32 precision)
                    dh = sb.tile([GR, N], bf16, tag=f"dh{g}")
                    nc.vector.tensor_copy(dh, d_sb)
                    r1 = sb.tile([GR, N], f32, tag="r1", bufs=2)
                    nc.vector.tensor_sub(r1, d_sb, dh)
                    dm = sb.tile([GR, N], bf16, tag=f"dm{g}")
                    nc.vector.tensor_copy(dm, r1)
                    r2 = sb.tile([GR, N], f32, tag="r2", bufs=2)
                    nc.vector.tensor_sub(r2, r1, dm)
                    dl = sb.tile([GR, N], bf16, tag=f"dl{g}")
                    nc.vector.tensor_copy(dl, r2)
                    split_tiles.append((dh, dm, dl))

            # ---- phase B: broadcast + gaussian + key-sum per row ----
            derf_func = AF.Derivative_Erf
            if func_override is not None:
                derf_func = func_override
            with tc.tile_pool(name="psB", bufs=1, space="PSUM") as psB:
                for g in range(NGROUPS):
                    # flatten the group's rows onto partitions 0..2:
                    # M_flat[c, a*N + j] = component c of row a at key j
                    M_flat = sb.tile([3, GR * N], bf16, tag="mflat", bufs=2)
                    for c, comp in enumerate(split_tiles[g]):
                        nc.gpsimd.dma_start(
                            out=M_flat[c : c + 1, :].rearrange(
                                "p (a j) -> p a j", a=GR
                            ),
                            in_=comp[:, :],
                        )
                    for a in range(0, GR, 4):
                        r = g * GR + a
                        # 4-row macro unit: two 2-row PSUM tiles -> one 4-row
                        # gsc tile; matmuls are three 512-col ops per 2 rows
                        # (row pairs are contiguous in M_flat, and 512-col
                        # windows align exactly with PSUM banks)
                        gsc = sb.tile([K, 4, N], f32, tag="gsc", bufs=3)
                        for h in range(2):
                            a2 = a + 2 * h
                            psum_db = psB.tile([K, 2, N], f32, tag="db", bufs=2)
                            flat = psum_db.rearrange("k a j -> k (a j)")
                            for lo in (0, 512, 1024):
                                nc.tensor.matmul(
                                    flat[:, lo : lo + 512],
                                    ones3,
                                    M_flat[0:3, a2 * N + lo : a2 * N + lo + 512],
                                    start=True, stop=True,
                                )
                            if not use_fallback:
                                nc.scalar.activation(
                                    out=gsc[:, 2 * h : 2 * h + 2, :],
                                    in_=psum_db,
                                    func=derf_func,
                                    bias=ebi_sb,
                                    scale=esc_sb,
                                )
                            else:
                                zsq = sb.tile([K, 2, N], f32, tag="zsq", bufs=3)
                                nc.scalar.activation(
                                    out=zsq, in_=psum_db,
                                    func=AF.Square, bias=ebi_sb, scale=esc_sb,
                                )
                                nc.scalar.activation(
                                    out=gsc[:, 2 * h : 2 * h + 2, :], in_=zsq,
                                    func=AF.Exp, bias=postc_sb, scale=-0.5,
                                )
                        # key-axis sum on DVE (4 rows per op)
                        nc.vector.reduce_sum(
                            out=S[:, r : r + 4], in_=gsc,
                            axis=mybir.AxisListType.X,
                        )

            # ---- phase C: channel constants + feature_proj MLP + output ----
            with tc.tile_pool(name="psC", bufs=1, space="PSUM") as psC:
                if not use_fallback:
                    nc.vector.tensor_scalar_mul(S, S, postc_sb)
                psum_h = psC.tile([K, RPC], f32, tag="mlp", bufs=2)
                nc.tensor.matmul(psum_h, w1_sb, S, start=True, stop=True)
                h_sb = sb.tile([K, RPC], f32, tag="h_sb")
                gelu_func = AF.Gelu if gelu_override is None else gelu_override
                nc.scalar.activation(h_sb, psum_h, gelu_func)
                o_sb = sb.tile([128, 2, RPC], f32, tag="o_sb")
                for e in range(2):
                    psum_o = psC.tile([128, RPC], f32, tag="mlp", bufs=2)
                    nc.tensor.matmul(
                        psum_o, w2_sb[:, 128 * e : 128 * (e + 1)], h_sb,
                        start=True, stop=True,
                    )
                    nc.vector.tensor_copy(o_sb[:, e, :], psum_o)
                for t in range(2):
                    out_sb = sb.tile([96, E], f32, tag=f"out{t}")
                    nc.gpsimd.dma_start(
                        out=out_sb, in_=rest[96 * t : 96 * (t + 1), :]
                    )
                    for e in range(2):
                        psum_t = psC.tile([96, 128], f32, tag="tr", bufs=2)
                        nc.tensor.transpose(
                            psum_t, o_sb[:, e, 96 * t : 96 * (t + 1)], id_sb
                        )
                        nc.vector.tensor_add(
                            out_sb[:, 128 * e : 128 * (e + 1)],
                            out_sb[:, 128 * e : 128 * (e + 1)],
                            psum_t,
                        )
                    nc.sync.dma_start(
                        out=out[96 * t : 96 * (t + 1), :], in_=out_sb
                    )

    nc.compile()
    return nc


# ---------------- host-side reference tails (numpy, f32) ----------------

def _erf_np(x):
    try:
        from scipy.special import erf
        return erf(x).astype(np.float32)
    except ImportError:
        f = np.frompyfunc(math.erf, 1, 1)
        return f(x.astype(np.float64)).astype(np.float32)


def _gelu_np(x):
    x = x.astype(np.float32)
    return (x * 0.5 * (1.0 + _erf_np(x / np.float32(math.sqrt(2.0))))).astype(
        np.float32
    )


def _silu_np(x):
    x = x.astype(np.float32)
    return (x / (1.0 + np.exp(-x))).astype(np.float32)


def _timestep_emb_np(t, dim):
    half = dim // 2
    freqs = np.exp(
        -np.log(10000.0) * np.arange(half, dtype=np.float32) / np.float32(half)
    ).astype(np.float32)
    a = t.astype(np.float32)[:, None] * freqs[None, :]
    return np.concatenate([np.sin(a), np.cos(a)], axis=-1).astype(np.float32)


def _host_tails(angle, mask_pos, time_pos, ang_w1, ang_w2, t_w1, t_b1, t_w2, t_b2):
    """rest[b, n, :] with rest[..., :INTER] = time_emb[..., :INTER] and
    rest[..., INTER:] = ang_f + time_emb[..., INTER:]."""
    angle = np.asarray(angle, np.float32)
    ang = np.where(np.isposinf(angle), np.float32(0.0), angle).astype(np.float32)
    ang_f = _gelu_np(ang @ np.asarray(ang_w1, np.float32)) @ np.asarray(
        ang_w2, np.float32
    )  # [B, N, INTER]

    def time_mlp(t):
        e = _timestep_emb_np(t, E)
        h = _silu_np(e @ np.asarray(t_w1, np.float32) + np.asarray(t_b1, np.float32))
        return (h @ np.asarray(t_w2, np.float32) + np.asarray(t_b2, np.float32)).astype(
            np.float32
        )

    tp = np.asarray(time_pos)
    te = time_mlp(tp)[:, None, :]                 # [B, 1, E]
    t0e = time_mlp(np.zeros_like(tp))[:, None, :]
    mask = np.asarray(mask_pos, bool)             # [B, N, 1]
    time_emb = np.where(mask, te, t0e).astype(np.float32)  # [B, N, E]

    rest = time_emb.copy()
    rest[..., INTER:] += ang_f.astype(np.float32)
    return rest.astype(np.float32)


def _prep_in_maps(pos, angle, padding_mask, mask_pos, time_pos,
                  means, stds, fp_w1, fp_w2, ang_w1, ang_w2,
                  t_w1, t_b1, t_w2, t_b2, use_fallback=None):
    if use_fallback is None:
        use_fallback = USE_FALLBACK_EXP
    pos = np.asarray(pos, np.float32)
    pad = np.asarray(padding_mask, bool)

    s = (np.abs(np.asarray(stds, np.float32)) + np.float32(0.01)).astype(np.float32)
    m = np.asarray(means, np.float32)
    inv_s = (np.float32(1.0) / s).astype(np.float32)
    if not use_fallback:
        # Derivative_Erf(x) with x = (d - m)/(s*sqrt(2))
        esc_v = (inv_s / np.float32(math.sqrt(2.0))).astype(np.float32)
        ebi_v = (-m * esc_v).astype(np.float32)
        postc_v = (
            np.float32(DERF_INV) / (np.float32(math.sqrt(2.0 * PI_REF)) * s)
        ).astype(np.float32)
    else:
        # Square then Exp(-0.5 z^2 + log c)
        esc_v = inv_s.astype(np.float32)
        ebi_v = (-m * inv_s).astype(np.float32)
        postc_v = np.log(
            np.float32(1.0) / (np.float32(math.sqrt(2.0 * PI_REF)) * s)
        ).astype(np.float32)

    blk3 = np.zeros((96, GR), np.float32)
    for p in range(96):
        blk3[p, p // 3] = 1.0

    rest = _host_tails(
        angle, mask_pos, time_pos, ang_w1, ang_w2, t_w1, t_b1, t_w2, t_b2
    )

    ident = np.eye(128, dtype=np.float32)
    w1_v = np.asarray(fp_w1, np.float32)
    w2_v = np.asarray(fp_w2, np.float32)

    in_maps = []
    for c in range(NCORES):
        b = c // (NCORES // B)
        r0 = (c % (NCORES // B)) * RPC
        posT = pos[b].T.copy()  # [3, N]
        if pad[b].any():
            posT[:, pad[b]] = np.float32(1.0e6)
        qscal = np.empty((96, NGROUPS), np.float32)
        for g in range(NGROUPS):
            rows = pos[b, r0 + g * GR : r0 + (g + 1) * GR, :]  # [32, 3]
            qscal[:, g] = rows.reshape(-1)
        in_maps.append(
            {
                "posT": np.ascontiguousarray(posT, np.float32),
                "qscal": qscal,
                "blk3": blk3,
                "esc": esc_v.reshape(K, 1),
                "ebi": ebi_v.reshape(K, 1),
                "postc": postc_v.reshape(K, 1),
                "w1": w1_v,
                "w2": w2_v,
                "ident": ident,
                "rest": np.ascontiguousarray(rest[b, r0 : r0 + RPC, :], np.float32),
            }
        )
    return in_maps


def kernel(pos, angle, node_type_edge, padding_mask, mask_aa, mask_pos, time_pos,
           means, stds, fp_w1, fp_w2, ang_w1, ang_w2, t_w1, t_b1, t_w2, t_b2):
    from concourse.bass_utils import run_bass_kernel_spmd

    key = ("nc", USE_FALLBACK_EXP, _FUNC_OVERRIDE)
    if key not in _COMPILED:
        _COMPILED[key] = _build_nc(func_override=_FUNC_OVERRIDE)
    nc = _COMPILED[key]

    in_maps = _prep_in_maps(
        pos, angle, padding_mask, mask_pos, time_pos, means, stds,
        fp_w1, fp_w2, ang_w1, ang_w2, t_w1, t_b1, t_w2, t_b2,
    )
    res = run_bass_kernel_spmd(nc, in_maps, core_ids=list(range(NCORES)))
    outs = [np.asarray(res.results[c]["out"], np.float32) for c in range(NCORES)]
    full = np.concatenate(outs, axis=0).reshape(B, N, E)
    return full
